# revision 16
# baseline (speedup 1.0000x reference)
"""Trainium2 Bass kernel for nn_AttentionLayerDecoder (sparse segment attention).

Math (reference, edge_index unused):
  qk[h,b,:]   = Wk[h] @ (context[b] @ Wq[h]) / 8          (tiny, host)
  u[h,n]      = x[n,:] . qk[h,batch[n],:]
  e[h,n]      = exp(u[h,n])                                (u ~ N(0,1))
  xe[h,b,:]   = sum_{n in b} e[h,n] * x[n,:]               (device)
  S[h,b]      = sum_{n in b} e[h,n]                        (device)
  out         = (qc*query + (xe @ Wv)/S) @ Wf, summed over heads  (tiny, host)

Device design (CoreSim cost-model driven):
  - A matmul is charged only for its OUTPUT free size (weight load is free),
    so both big contractions keep x as the *stationary* operand:
      u:  out[node,h]  = lhsT(x^T [f,node]) ^T @ qk[f,h]     -> 8 rows/tile
      xe: out[f,h]    += lhsT(x   [node,f]) ^T @ e[node,h]   -> 8 rows/tile
  - DMA transfer bytes are charged to the issuing queue; SP / Activation
    (HWDGE) / Pool (SWDGE) run in parallel. x ships in fp8 (rel err 3e-3 <<
    2e-2 budget) from one flat f-major tensor XALL = [qkt | packed | natural].
  - x is needed in BOTH layouts. Small ("natural") graphs get a second fp8
    DMA copy (node-major, tensor XN). The biggest K_PACK graphs ("packed",
    single DMA copy) instead build it on-chip: an fp16-viewed PE transpose
    moves TWO fp8 nodes per row, DVE evacuates batches, and the xe matmul
    reads a stride-2 fp8 view with even/odd parities as two matmuls.
  - Graphs are sorted by node count per core and every slot is sized to the
    cross-core max (rounded to 128 nodes natural / 256 packed) to cut the
    padding bytes; the module is cached per size-signature.
  - exp on ACT in <=512-column batches (packed group first); only tiny
    [128,8]-per-graph results leave PSUM; the final two evacuations run in
    parallel on ACT and DVE so only one copy + one DMA sit on the tail.
"""

import sys

if "/opt/trn_rl_repo" not in sys.path:
    sys.path.insert(0, "/opt/trn_rl_repo")

from contextlib import ExitStack

import ml_dtypes
import numpy as np

import concourse.bass as bass
import concourse.tile as tile
from concourse import bacc, masks, mybir
from concourse.bass_utils import run_bass_kernel_spmd

N_CORES = 8
H = 8          # heads
F = 128        # node feature dim
B = 128        # graphs
GPC = B // N_CORES  # graphs per core

K_PACK = 6             # graphs whose node-major copy is built by PE transpose
NAT = GPC - K_PACK     # graphs that get a second (node-major) DMA copy

FP8 = ml_dtypes.float8_e4m3

_CACHE = {}


def _build(sizes):
    """sizes[g] = padded node count of slot g (mult of 128 for g < NAT,
    mult of 256 for packed slots g >= NAT)."""
    nc = bacc.Bacc(None, target_bir_lowering=False)
    fp8 = mybir.dt.float8e4
    fp16 = mybir.dt.float16
    f32 = mybir.dt.float32
    AFT = mybir.ActivationFunctionType

    NT = [sizes[g] // 128 for g in range(GPC)]       # 128-tiles (natural use)
    QPs = [sizes[g] // 256 for g in range(GPC)]      # 256-groups (packed use)
    ewid = [
        (QPs[g] * 2 * H if g >= NAT else NT[g] * H) for g in range(GPC)
    ]

    xo, off = [], 256
    for g in range(GPC):
        xo.append(off)
        off += sizes[g]
    W = off                                          # XALL width
    xno, off = [], 0
    for g in range(NAT):
        xno.append(off)
        off += NT[g] * F
    XNW = max(off, 1)

    XALL = nc.dram_tensor("XALL", [F, W], fp8, kind="ExternalInput")
    XN = nc.dram_tensor("XN", [128, XNW], fp8, kind="ExternalInput")
    OUT = nc.dram_tensor("OUT", [128, GPC * H + GPC], f32, kind="ExternalOutput")

    e_off, off = [], 0
    for g in range(GPC):
        e_off.append(off)
        off += ewid[g]
    e_tot = off
    # exp batches: packed-graph group first (its slabs arrive earliest)
    groups, cur, w = [], [], 0
    for g in range(GPC - 1, -1, -1):
        if w + ewid[g] > 512:
            groups.append(cur)
            cur, w = [], 0
        cur.insert(0, g)
        w += ewid[g]
    groups.append(cur)

    # transpose/evac batches over packed slots: first single, then pairs
    tbatch = []
    if K_PACK > 0:
        gs = list(range(NAT, GPC))
        tbatch.append([gs[0]])
        i = 1
        while i < len(gs):
            tbatch.append(gs[i : i + 2])
            i += 2

    with tile.TileContext(nc) as tc, ExitStack() as ctx:
        const = ctx.enter_context(tc.tile_pool(name="const", bufs=1))
        xpool = ctx.enter_context(tc.tile_pool(name="x", bufs=1))
        epool = ctx.enter_context(tc.tile_pool(name="e", bufs=1))
        outp = ctx.enter_context(tc.tile_pool(name="outp", bufs=1))
        ps_u = ctx.enter_context(
            tc.tile_pool(name="ps_u", bufs=2, space=bass.MemorySpace.PSUM)
        )
        ps_t = ctx.enter_context(
            tc.tile_pool(name="ps_t", bufs=3, space=bass.MemorySpace.PSUM)
        )
        ps_o = ctx.enter_context(
            tc.tile_pool(name="ps_o", bufs=1, space=bass.MemorySpace.PSUM)
        )

        ones = const.tile([128, 1], fp16)
        warm = const.tile([1, 1], f32)
        warm2 = const.tile([1, 1], fp16)
        ident = const.tile([128, 128], fp16)
        out_sb = outp.tile([128, GPC * H + GPC], f32)
        nc.gpsimd.memset(ones[:], 1.0)
        nc.gpsimd.memset(warm[:], 0.0)
        masks.make_identity(nc, ident[:])
        # Warm the ACT exp table at t~0 (1283ns table load hides under DMA).
        nc.scalar.activation(warm2[:], warm[:], AFT.Exp)

        xall = xpool.tile([128, W], fp8)
        xn_all = xpool.tile([128, XNW], fp8)
        xnp = {}   # packed slot -> (sbuf tile, col offset)
        tb_tiles = []
        for bi, gs in enumerate(tbatch):
            cols = sum(QPs[g] * 128 for g in gs)
            t_sb = xpool.tile([128, cols], fp16, name=f"xnp{bi}")
            tb_tiles.append(t_sb)
            o = 0
            for g in gs:
                xnp[g] = (t_sb, o)
                o += QPs[g] * 128
        e_all = epool.tile([128, e_tot], fp16)
        qkt = xall[:, 0:256].bitcast(fp16)   # [128, GPC*H]

        # ---- DMA schedule: greedy; XN (tail) chunks avoid the Pool queue ----
        qload = {"sp": 500.0, "pool": 650.0, "act": 1283.0}
        qdelay = {"sp": 1716.0, "pool": 1883.0, "act": 1716.0}
        qeng = {"sp": nc.sync, "pool": nc.gpsimd, "act": nc.scalar}

        def pick(extra_delay=False):
            if extra_delay:
                return min(qload, key=lambda k: qload[k] + qdelay[k])
            return min(qload, key=lambda k: qload[k])

        def issue_cols(a, b):
            q = pick()
            qload[q] += max((b - a) * 0.3855, 500.0)
            qeng[q].dma_start(xall[:, a:b], XALL[:, a:b])

        # chunk 0: qkt + first packed slab (small -> transposes start early);
        # then packed pairs, natural slabs (exp-group order), XN chunks last.
        if K_PACK > 0:
            issue_cols(0, xo[NAT] + sizes[NAT])
            for j in range(NAT + 1, GPC, 2):
                b = min(j + 2, GPC)
                issue_cols(xo[j], xo[b - 1] + sizes[b - 1])
        else:
            issue_cols(0, 256)
        nat_order = [g for grp in groups for g in grp if g < NAT]
        for i in range(0, len(nat_order), 2):
            gs = sorted(nat_order[i : i + 2])
            if len(gs) == 2 and xo[gs[1]] == xo[gs[0]] + sizes[gs[0]]:
                issue_cols(xo[gs[0]], xo[gs[1]] + sizes[gs[1]])
            else:
                for g in gs:
                    issue_cols(xo[g], xo[g] + sizes[g])
        for c in range(0, NAT, 2):
            n = min(2, NAT - c)
            a, b = xno[c], xno[c + n - 1] + NT[c + n - 1] * F
            q = pick(extra_delay=True)
            qload[q] += max((b - a) * 0.3855, 500.0)
            qeng[q].dma_start(xn_all[:, a:b], XN[:, a:b])

        # ---- PE stream: transposes first (earliest data), then u, S, xe ----
        for bi, gs in enumerate(tbatch):
            cols = sum(QPs[g] * 128 for g in gs)
            tp = ps_t.tile([128, cols], fp16, tag="tp", name=f"tp{bi}")
            o = 0
            for g in gs:
                sl = xall[:, xo[g] : xo[g] + sizes[g]].bitcast(fp16)
                for q in range(QPs[g]):
                    nc.tensor.transpose(
                        tp[:, o + q * 128 : o + (q + 1) * 128],
                        sl[:, q * 128 : (q + 1) * 128],
                        ident[:],
                    )
                o += QPs[g] * 128
            nc.vector.tensor_copy(tb_tiles[bi][:], tp[:])

        def u_graph(g, u_ps, base):
            goff = xo[g]
            if g >= NAT:
                for q in range(QPs[g]):
                    v = xall[:, goff + q * 256 : goff + (q + 1) * 256].rearrange(
                        "f (n two) -> f two n", two=2
                    )
                    for par in range(2):
                        c0 = base + (q * 2 + par) * H
                        nc.tensor.matmul(
                            u_ps[:, c0 : c0 + H],
                            v[:, par, :],
                            qkt[:, g * H : (g + 1) * H],
                            start=True,
                            stop=True,
                        )
            else:
                for t in range(NT[g]):
                    nc.tensor.matmul(
                        u_ps[:, base + t * H : base + (t + 1) * H],
                        xall[:, goff + t * 128 : goff + (t + 1) * 128],
                        qkt[:, g * H : (g + 1) * H],
                        start=True,
                        stop=True,
                    )

        # S + last natural graph's xe share one PSUM tile (bank budget):
        # combo cols 0:H = xe of slot NAT-1, cols H:H+GPC = S per slot.
        xe_a = ps_o.tile([128, (GPC - 1) * H], f32)
        combo = ps_o.tile([128, H + GPC], f32)
        nc.vector.memset(combo[:], 0.0)

        for grp in groups:
            w = sum(ewid[g] for g in grp)
            u_ps = ps_u.tile([128, w], f32, tag="u", name=f"u{grp[0]}")
            base = 0
            for g in grp:
                u_graph(g, u_ps, base)
                base += ewid[g]
            nc.scalar.activation(
                e_all[:, e_off[grp[0]] : e_off[grp[0]] + w], u_ps[:], AFT.Exp
            )
            for g in grp:   # S matmuls as soon as this batch's e is ready
                nc.tensor.matmul(
                    combo[0 : ewid[g], H + g : H + g + 1],
                    e_all[:, e_off[g] : e_off[g] + ewid[g]],
                    ones[:],
                    start=True,
                    stop=True,
                )

        def xcol(g):   # xe_a column of slot g (slot NAT-1 lives in combo)
            return g if g < NAT - 1 else g - 1

        xe_order = list(range(NAT, GPC)) + list(range(NAT))
        for g in xe_order:
            dst = (
                combo[:, 0:H]
                if g == NAT - 1
                else xe_a[:, xcol(g) * H : (xcol(g) + 1) * H]
            )
            if g >= NAT:
                t_sb, o = xnp[g]
                for q in range(QPs[g]):
                    v = (
                        t_sb[:, o + q * 128 : o + (q + 1) * 128]
                        .bitcast(fp8)
                        .rearrange("p (f two) -> p two f", two=2)
                    )
                    for par in range(2):
                        nc.tensor.matmul(
                            dst,
                            v[:, par, :],
                            e_all[
                                :,
                                e_off[g] + (q * 2 + par) * H : e_off[g]
                                + (q * 2 + par + 1) * H,
                            ],
                            start=(q == 0 and par == 0),
                            stop=(q == QPs[g] - 1 and par == 1),
                        )
            else:
                for t in range(NT[g]):
                    nc.tensor.matmul(
                        dst,
                        xn_all[:, xno[g] + t * F : xno[g] + (t + 1) * F],
                        e_all[:, e_off[g] + t * H : e_off[g] + (t + 1) * H],
                        start=(t == 0),
                        stop=(t == NT[g] - 1),
                    )
            if g == NAT - 2:
                # bulk evacuation on ACT, in parallel with DVE's tail combo
                nc.scalar.copy(out_sb[:, 0 : (GPC - 1) * H], xe_a[:])
        nc.vector.tensor_copy(out_sb[:, (GPC - 1) * H :], combo[:])
        nc.sync.dma_start(OUT[:], out_sb[:])

    nc.compile()
    return nc


def _get(sizes):
    key = tuple(sizes)
    if key not in _CACHE:
        _CACHE[key] = _build(list(key))
    return _CACHE[key]


def _prepare(x, batch, context, Wq, Wk):
    """Host-side shard prep. Returns (in_maps, sizes, order, query, n_pad)."""
    counts = np.bincount(batch, minlength=B).astype(np.int64)
    starts = np.zeros(B + 1, np.int64)
    np.cumsum(counts, out=starts[1:])

    # slot assignment: per core, sort graphs ascending by count; slot sizes
    # are the cross-core max, rounded to 128 (natural) / 256 (packed slots,
    # which hold the biggest graphs and ship only one copy).
    order = np.zeros((N_CORES, GPC), np.int64)     # slot -> local graph idx
    slotc = np.zeros((N_CORES, GPC), np.int64)
    for c in range(N_CORES):
        cc = counts[c * GPC : (c + 1) * GPC]
        o = np.argsort(cc, kind="stable")
        order[c] = o
        slotc[c] = cc[o]
    mx = slotc.max(axis=0)
    sizes = []
    for g in range(GPC):
        m = 256 if g >= NAT else 128
        sizes.append(int(-(-int(mx[g]) // m) * m))

    query = np.einsum("bc,hcv->hbv", context, Wq).astype(np.float32)  # [H,B,Dv]
    qk = np.einsum("hbv,hev->hbe", query, Wk).astype(np.float32)      # [H,B,F]
    qk8 = (qk / 8.0).astype(np.float16)

    x8 = x.astype(FP8)

    xo, off = [], 256
    for g in range(GPC):
        xo.append(off)
        off += sizes[g]
    Wall = off
    xno, off = [], 0
    for g in range(NAT):
        xno.append(off)
        off += (sizes[g] // 128) * F
    XNW = max(off, 1)

    n_pad = np.zeros(B, np.float32)
    in_maps = []
    for c in range(N_CORES):
        XALLc = np.zeros((F, Wall), FP8)
        XNc = np.zeros((128, XNW), FP8)
        QKTc = np.zeros((F, GPC * H), np.float16)
        for g in range(GPC):
            gi = int(order[c, g])
            b = c * GPC + gi
            n0, n1 = int(starts[b]), int(starts[b + 1])
            cnt = n1 - n0
            XALLc[:, xo[g] : xo[g] + cnt] = x8[n0:n1].T
            n_pad[b] = sizes[g] - cnt
            if g < NAT:
                T = sizes[g] // 128
                buf = np.zeros((T * 128, F), FP8)
                buf[0:cnt] = x8[n0:n1]
                XNc[:, xno[g] : xno[g] + T * F] = (
                    buf.reshape(T, 128, F).transpose(1, 0, 2).reshape(128, T * F)
                )
            QKTc[:, g * H : (g + 1) * H] = qk8[:, b, :].T
        XALLc[:, 0:256] = QKTc.view(FP8)
        in_maps.append({"XALL": XALLc, "XN": XNc})
    return in_maps, sizes, order, query, n_pad


def kernel(**inputs):
    x = np.asarray(inputs["x"], np.float32)
    batch = np.asarray(inputs["batch"]).astype(np.int64)
    context = np.asarray(inputs["context"], np.float32)
    Wq = np.asarray(inputs["Wq"], np.float32)
    Wk = np.asarray(inputs["Wk"], np.float32)
    Wv = np.asarray(inputs["Wv"], np.float32)
    qc = float(np.asarray(inputs["query_coef"]).reshape(-1)[0])
    Wf = np.asarray(inputs["Wf"], np.float32)

    in_maps, sizes, order, query, n_pad = _prepare(x, batch, context, Wq, Wk)

    nc = _get(sizes)
    res = run_bass_kernel_spmd(nc, in_maps, core_ids=list(range(N_CORES)))

    XE = np.zeros((H, B, F), np.float32)
    S = np.zeros((H, B), np.float32)
    slot_of = [g if g < NAT - 1 else g - 1 for g in range(GPC)]
    slot_of[NAT - 1] = GPC - 1
    for c in range(N_CORES):
        out = res.results[c]["OUT"]                      # [128, GPC*H + GPC]
        xeT = out[:, 0 : GPC * H].reshape(F, GPC, H)     # [f, slot, h]
        s = out[:, GPC * H :]
        for g in range(GPC):
            gi = int(order[c, g])
            b = c * GPC + gi
            XE[:, b, :] = xeT[:, slot_of[g], :].T
            w = (sizes[g] // 256) * 2 if g >= NAT else sizes[g] // 128
            S[:, b] = s[0 : w * H, g].reshape(w, H).sum(axis=0)

    S = S - n_pad[None, :]  # pad slots contributed exp(0)*1 each to S
    Y = np.einsum("hbe,hev->hbv", XE, Wv.astype(np.float32))
    agg = Y / (S[..., None] + 1e-16)
    hbv = qc * query + agg
    out = np.einsum("hbv,ve->be", hbv, Wf)
    return out.astype(np.float32)


# revision 17
# speedup vs baseline: 1.5417x; 1.5417x over previous
"""Trainium2 Bass kernel for nn_AttentionLayerDecoder (sparse segment attention).

Math (reference, edge_index unused):
  qk[h,b,:]   = Wk[h] @ (context[b] @ Wq[h]) / 8          (tiny, host)
  u[h,n]      = x[n,:] . qk[h,batch[n],:]
  e[h,n]      = exp(u[h,n])                                (u ~ N(0,1))
  xe[h,b,:]   = sum_{n in b} e[h,n] * x[n,:]               (device)
  S[h,b]      = sum_{n in b} e[h,n]                        (device)
  out         = (qc*query + (xe @ Wv)/S) @ Wf, summed over heads  (tiny, host)

Device design (CoreSim cost-model driven):
  - A matmul is charged only for its OUTPUT free size (weight load is free),
    so both big contractions keep x as the *stationary* operand:
      u:  out[node,h]  = lhsT(x^T [f,node]) ^T @ qk[f,h]     -> 8 rows/tile
      xe: out[f,h]    += lhsT(x   [node,f]) ^T @ e[node,h]   -> 8 rows/tile
  - DMA transfer bytes are charged to the issuing queue; SP / Activation
    (HWDGE) / Pool (SWDGE) run in parallel. x ships in fp8 (rel err 3e-3 <<
    2e-2 budget) from one flat f-major tensor XALL = [qkt | packed | natural].
  - x is needed in BOTH layouts. Small ("natural") graphs get a second fp8
    DMA copy (node-major, tensor XN). The biggest K_PACK graphs ("packed",
    single DMA copy) instead build it on-chip: an fp16-viewed PE transpose
    moves TWO fp8 nodes per row, DVE evacuates batches, and the xe matmul
    reads a stride-2 fp8 view with even/odd parities as two matmuls.
  - Graphs are sorted by node count per core and every slot is sized to the
    cross-core max (rounded to 128 nodes natural / 256 packed) to cut the
    padding bytes; the module is cached per size-signature.
  - exp on ACT in <=512-column batches (packed group first); only tiny
    [128,8]-per-graph results leave PSUM; the final two evacuations run in
    parallel on ACT and DVE so only one copy + one DMA sit on the tail.
"""

import sys

if "/opt/trn_rl_repo" not in sys.path:
    sys.path.insert(0, "/opt/trn_rl_repo")

from contextlib import ExitStack

import ml_dtypes
import numpy as np

import concourse.bass as bass
import concourse.tile as tile
from concourse import bacc, masks, mybir
from concourse.bass_utils import run_bass_kernel_spmd

N_CORES = 8
H = 8          # heads
F = 128        # node feature dim
B = 128        # graphs
GPC = B // N_CORES  # graphs per core

K_PACK = 6             # graphs whose node-major copy is built by PE transpose
NAT = GPC - K_PACK     # graphs that get a second (node-major) DMA copy

FP8 = ml_dtypes.float8_e4m3

_CACHE = {}


def _build(sizes):
    """sizes[g] = padded node count of slot g (mult of 128 for g < NAT,
    mult of 256 for packed slots g >= NAT)."""
    nc = bacc.Bacc(None, target_bir_lowering=False)
    fp8 = mybir.dt.float8e4
    fp16 = mybir.dt.float16
    f32 = mybir.dt.float32
    AFT = mybir.ActivationFunctionType

    NT = [sizes[g] // 128 for g in range(GPC)]       # 128-tiles (natural use)
    QPs = [sizes[g] // 256 for g in range(GPC)]      # 256-groups (packed use)
    ewid = [
        (QPs[g] * 2 * H if g >= NAT else NT[g] * H) for g in range(GPC)
    ]

    # XALL column layout: [qkt | packed slots NAT..GPC-1 | natural slots]
    xo = [0] * GPC
    off = 256
    for g in list(range(NAT, GPC)) + list(range(NAT)):
        xo[g] = off
        off += sizes[g]
    W = off                                          # XALL width
    xno, off = [], 0
    for g in range(NAT):
        xno.append(off)
        off += NT[g] * F
    XNW = max(off, 1)

    XALL = nc.dram_tensor("XALL", [F, W], fp8, kind="ExternalInput")
    XN = nc.dram_tensor("XN", [128, XNW], fp8, kind="ExternalInput")
    OUT = nc.dram_tensor("OUT", [128, GPC * H + GPC], f32, kind="ExternalOutput")

    e_off, off = [], 0
    for g in range(GPC):
        e_off.append(off)
        off += ewid[g]
    e_tot = off
    # exp batches: packed-graph group first (its slabs arrive earliest)
    groups, cur, w = [], [], 0
    for g in range(GPC - 1, -1, -1):
        if w + ewid[g] > 512:
            groups.append(cur)
            cur, w = [], 0
        cur.insert(0, g)
        w += ewid[g]
    groups.append(cur)

    # transpose/evac batches over packed slots: first single, then pairs
    tbatch = []
    if K_PACK > 0:
        gs = list(range(NAT, GPC))
        tbatch.append([gs[0]])
        i = 1
        while i < len(gs):
            tbatch.append(gs[i : i + 2])
            i += 2

    with tile.TileContext(nc) as tc, ExitStack() as ctx:
        const = ctx.enter_context(tc.tile_pool(name="const", bufs=1))
        xpool = ctx.enter_context(tc.tile_pool(name="x", bufs=1))
        epool = ctx.enter_context(tc.tile_pool(name="e", bufs=1))
        outp = ctx.enter_context(tc.tile_pool(name="outp", bufs=1))
        ps_u = ctx.enter_context(
            tc.tile_pool(name="ps_u", bufs=2, space=bass.MemorySpace.PSUM)
        )
        ps_t = ctx.enter_context(
            tc.tile_pool(name="ps_t", bufs=3, space=bass.MemorySpace.PSUM)
        )
        ps_o = ctx.enter_context(
            tc.tile_pool(name="ps_o", bufs=1, space=bass.MemorySpace.PSUM)
        )

        ones = const.tile([128, 1], fp16)
        warm = const.tile([1, 1], f32)
        warm2 = const.tile([1, 1], fp16)
        ident = const.tile([128, 128], fp16)
        out_sb = outp.tile([128, GPC * H + GPC], f32)
        nc.gpsimd.memset(ones[:], 1.0)
        nc.gpsimd.memset(warm[:], 0.0)
        masks.make_identity(nc, ident[:])
        # Warm the ACT exp table at t~0 (1283ns table load hides under DMA).
        nc.scalar.activation(warm2[:], warm[:], AFT.Exp)

        xall = xpool.tile([128, W], fp8)
        xn_all = xpool.tile([128, XNW], fp8)
        xnp = {}   # packed slot -> (sbuf tile, col offset)
        tb_tiles = []
        for bi, gs in enumerate(tbatch):
            cols = sum(QPs[g] * 128 for g in gs)
            t_sb = xpool.tile([128, cols], fp16, name=f"xnp{bi}")
            tb_tiles.append(t_sb)
            o = 0
            for g in gs:
                xnp[g] = (t_sb, o)
                o += QPs[g] * 128
        e_all = epool.tile([128, e_tot], fp16)
        qkt = xall[:, 0:256].bitcast(fp16)   # [128, GPC*H]

        # ---- DMA schedule: greedy; XN (tail) chunks avoid the Pool queue ----
        qload = {"sp": 500.0, "pool": 650.0, "act": 1283.0}
        qdelay = {"sp": 1716.0, "pool": 1883.0, "act": 1716.0}
        qeng = {"sp": nc.sync, "pool": nc.gpsimd, "act": nc.scalar}

        def pick(extra_delay=False):
            if extra_delay:
                return min(qload, key=lambda k: qload[k] + qdelay[k])
            return min(qload, key=lambda k: qload[k])

        def issue_cols(a, b):
            q = pick()
            qload[q] += max((b - a) * 0.3855, 500.0)
            qeng[q].dma_start(xall[:, a:b], XALL[:, a:b])

        # chunk 0: qkt + first packed slab (small -> transposes start early);
        # then packed pairs, natural slabs (exp-group order), XN chunks last.
        if K_PACK > 0:
            issue_cols(0, xo[NAT] + sizes[NAT])
            for j in range(NAT + 1, GPC, 2):
                b = min(j + 2, GPC)
                issue_cols(xo[j], xo[b - 1] + sizes[b - 1])
        else:
            issue_cols(0, 256)
        nat_order = [g for grp in groups for g in grp if g < NAT]
        for i in range(0, len(nat_order), 2):
            gs = sorted(nat_order[i : i + 2])
            if len(gs) == 2 and xo[gs[1]] == xo[gs[0]] + sizes[gs[0]]:
                issue_cols(xo[gs[0]], xo[gs[1]] + sizes[gs[1]])
            else:
                for g in gs:
                    issue_cols(xo[g], xo[g] + sizes[g])
        for c in range(0, NAT, 2):
            n = min(2, NAT - c)
            a, b = xno[c], xno[c + n - 1] + NT[c + n - 1] * F
            q = pick(extra_delay=True)
            qload[q] += max((b - a) * 0.3855, 500.0)
            qeng[q].dma_start(xn_all[:, a:b], XN[:, a:b])

        # ---- PE stream: transposes first (earliest data), then u, S, xe ----
        for bi, gs in enumerate(tbatch):
            cols = sum(QPs[g] * 128 for g in gs)
            tp = ps_t.tile([128, cols], fp16, tag="tp", name=f"tp{bi}")
            o = 0
            for g in gs:
                sl = xall[:, xo[g] : xo[g] + sizes[g]].bitcast(fp16)
                for q in range(QPs[g]):
                    nc.tensor.transpose(
                        tp[:, o + q * 128 : o + (q + 1) * 128],
                        sl[:, q * 128 : (q + 1) * 128],
                        ident[:],
                    )
                o += QPs[g] * 128
            nc.vector.tensor_copy(tb_tiles[bi][:], tp[:])

        def u_graph(g, u_ps, base):
            goff = xo[g]
            if g >= NAT:
                for q in range(QPs[g]):
                    v = xall[:, goff + q * 256 : goff + (q + 1) * 256].rearrange(
                        "f (n two) -> f two n", two=2
                    )
                    for par in range(2):
                        c0 = base + (q * 2 + par) * H
                        nc.tensor.matmul(
                            u_ps[:, c0 : c0 + H],
                            v[:, par, :],
                            qkt[:, g * H : (g + 1) * H],
                            start=True,
                            stop=True,
                        )
            else:
                for t in range(NT[g]):
                    nc.tensor.matmul(
                        u_ps[:, base + t * H : base + (t + 1) * H],
                        xall[:, goff + t * 128 : goff + (t + 1) * 128],
                        qkt[:, g * H : (g + 1) * H],
                        start=True,
                        stop=True,
                    )

        # S + last natural graph's xe share one PSUM tile (bank budget):
        # combo cols 0:H = xe of slot NAT-1, cols H:H+GPC = S per slot.
        xe_a = ps_o.tile([128, (GPC - 1) * H], f32)
        combo = ps_o.tile([128, H + GPC], f32)
        nc.vector.memset(combo[:], 0.0)

        for grp in groups:
            w = sum(ewid[g] for g in grp)
            u_ps = ps_u.tile([128, w], f32, tag="u", name=f"u{grp[0]}")
            base = 0
            for g in grp:
                u_graph(g, u_ps, base)
                base += ewid[g]
            nc.scalar.activation(
                e_all[:, e_off[grp[0]] : e_off[grp[0]] + w], u_ps[:], AFT.Exp
            )
            for g in grp:   # S matmuls as soon as this batch's e is ready
                nc.tensor.matmul(
                    combo[0 : ewid[g], H + g : H + g + 1],
                    e_all[:, e_off[g] : e_off[g] + ewid[g]],
                    ones[:],
                    start=True,
                    stop=True,
                )

        def xcol(g):   # xe_a column of slot g (slot NAT-1 lives in combo)
            return g if g < NAT - 1 else g - 1

        xe_order = list(range(NAT, GPC)) + list(range(NAT))
        for g in xe_order:
            dst = (
                combo[:, 0:H]
                if g == NAT - 1
                else xe_a[:, xcol(g) * H : (xcol(g) + 1) * H]
            )
            if g >= NAT:
                t_sb, o = xnp[g]
                for q in range(QPs[g]):
                    v = (
                        t_sb[:, o + q * 128 : o + (q + 1) * 128]
                        .bitcast(fp8)
                        .rearrange("p (f two) -> p two f", two=2)
                    )
                    for par in range(2):
                        nc.tensor.matmul(
                            dst,
                            v[:, par, :],
                            e_all[
                                :,
                                e_off[g] + (q * 2 + par) * H : e_off[g]
                                + (q * 2 + par + 1) * H,
                            ],
                            start=(q == 0 and par == 0),
                            stop=(q == QPs[g] - 1 and par == 1),
                        )
            else:
                for t in range(NT[g]):
                    nc.tensor.matmul(
                        dst,
                        xn_all[:, xno[g] + t * F : xno[g] + (t + 1) * F],
                        e_all[:, e_off[g] + t * H : e_off[g] + (t + 1) * H],
                        start=(t == 0),
                        stop=(t == NT[g] - 1),
                    )
            if g == NAT - 2:
                # bulk evacuation on ACT, in parallel with DVE's tail combo
                nc.scalar.copy(out_sb[:, 0 : (GPC - 1) * H], xe_a[:])
        nc.vector.tensor_copy(out_sb[:, (GPC - 1) * H :], combo[:])
        nc.sync.dma_start(OUT[:], out_sb[:])

    nc.compile()
    return nc


def _get(sizes):
    key = tuple(sizes)
    if key not in _CACHE:
        _CACHE[key] = _build(list(key))
    return _CACHE[key]


def _prepare(x, batch, context, Wq, Wk):
    """Host-side shard prep. Returns (in_maps, sizes, order, query, n_pad)."""
    counts = np.bincount(batch, minlength=B).astype(np.int64)
    starts = np.zeros(B + 1, np.int64)
    np.cumsum(counts, out=starts[1:])

    # slot assignment: per core, sort graphs ascending by count; slot sizes
    # are the cross-core max, rounded to 128 (natural) / 256 (packed slots,
    # which hold the biggest graphs and ship only one copy).
    order = np.zeros((N_CORES, GPC), np.int64)     # slot -> local graph idx
    slotc = np.zeros((N_CORES, GPC), np.int64)
    for c in range(N_CORES):
        cc = counts[c * GPC : (c + 1) * GPC]
        o = np.argsort(cc, kind="stable")
        order[c] = o
        slotc[c] = cc[o]
    mx = slotc.max(axis=0)
    sizes = []
    for g in range(GPC):
        m = 256 if g >= NAT else 128
        sizes.append(int(-(-int(mx[g]) // m) * m))

    query = np.einsum("bc,hcv->hbv", context, Wq).astype(np.float32)  # [H,B,Dv]
    qk = np.einsum("hbv,hev->hbe", query, Wk).astype(np.float32)      # [H,B,F]
    qk8 = (qk / 8.0).astype(np.float16)

    x8 = x.astype(FP8)

    xo = [0] * GPC
    off = 256
    for g in list(range(NAT, GPC)) + list(range(NAT)):
        xo[g] = off
        off += sizes[g]
    Wall = off
    xno, off = [], 0
    for g in range(NAT):
        xno.append(off)
        off += (sizes[g] // 128) * F
    XNW = max(off, 1)

    n_pad = np.zeros(B, np.float32)
    in_maps = []
    for c in range(N_CORES):
        XALLc = np.zeros((F, Wall), FP8)
        XNc = np.zeros((128, XNW), FP8)
        QKTc = np.zeros((F, GPC * H), np.float16)
        for g in range(GPC):
            gi = int(order[c, g])
            b = c * GPC + gi
            n0, n1 = int(starts[b]), int(starts[b + 1])
            cnt = n1 - n0
            XALLc[:, xo[g] : xo[g] + cnt] = x8[n0:n1].T
            n_pad[b] = sizes[g] - cnt
            if g < NAT:
                T = sizes[g] // 128
                buf = np.zeros((T * 128, F), FP8)
                buf[0:cnt] = x8[n0:n1]
                XNc[:, xno[g] : xno[g] + T * F] = (
                    buf.reshape(T, 128, F).transpose(1, 0, 2).reshape(128, T * F)
                )
            QKTc[:, g * H : (g + 1) * H] = qk8[:, b, :].T
        XALLc[:, 0:256] = QKTc.view(FP8)
        in_maps.append({"XALL": XALLc, "XN": XNc})
    return in_maps, sizes, order, query, n_pad


def kernel(**inputs):
    x = np.asarray(inputs["x"], np.float32)
    batch = np.asarray(inputs["batch"]).astype(np.int64)
    context = np.asarray(inputs["context"], np.float32)
    Wq = np.asarray(inputs["Wq"], np.float32)
    Wk = np.asarray(inputs["Wk"], np.float32)
    Wv = np.asarray(inputs["Wv"], np.float32)
    qc = float(np.asarray(inputs["query_coef"]).reshape(-1)[0])
    Wf = np.asarray(inputs["Wf"], np.float32)

    in_maps, sizes, order, query, n_pad = _prepare(x, batch, context, Wq, Wk)

    nc = _get(sizes)
    res = run_bass_kernel_spmd(nc, in_maps, core_ids=list(range(N_CORES)))

    XE = np.zeros((H, B, F), np.float32)
    S = np.zeros((H, B), np.float32)
    slot_of = [g if g < NAT - 1 else g - 1 for g in range(GPC)]
    slot_of[NAT - 1] = GPC - 1
    for c in range(N_CORES):
        out = res.results[c]["OUT"]                      # [128, GPC*H + GPC]
        xeT = out[:, 0 : GPC * H].reshape(F, GPC, H)     # [f, slot, h]
        s = out[:, GPC * H :]
        for g in range(GPC):
            gi = int(order[c, g])
            b = c * GPC + gi
            XE[:, b, :] = xeT[:, slot_of[g], :].T
            w = (sizes[g] // 256) * 2 if g >= NAT else sizes[g] // 128
            S[:, b] = s[0 : w * H, g].reshape(w, H).sum(axis=0)

    S = S - n_pad[None, :]  # pad slots contributed exp(0)*1 each to S
    Y = np.einsum("hbe,hev->hbv", XE, Wv.astype(np.float32))
    agg = Y / (S[..., None] + 1e-16)
    hbv = qc * query + agg
    out = np.einsum("hbv,ve->be", hbv, Wf)
    return out.astype(np.float32)
